# revision 33
# baseline (speedup 1.0000x reference)
"""Trainium2 Bass kernel for nn_CSG2A_net (gnn_message_passing).

Math (algebraically identical to the reference, never materializes the
[B,G,G] score tensor):
  CCE:  h = relu(node_feat @ W1); w = adj*exp(-dist)
        g[b,m] = sum_n mask[b,n] * w[b,n,m]
        pooled[b,d] = (sum_m g[b,m] h[b,m,d]) / clip(sum_n mask[b,n], 1)
        comp = pooled @ W2 + dose @ w_dose + time @ w_time
  score.sum(-1)[b,g] = q[b,g,:] . (sum_k q[b,k,:]) / sqrt(H)
    with q[b,g,:] = b_gex[b,g] w_gex[g,:] + comp[b,g] w_comp[g,:]
    so  u = (b_gex @ w_gex + comp @ w_comp) / sqrt(H)   [H,B]
        A = w_gex @ u ; C = w_comp @ u                  [G,B]
        ssum = b_gex*A + comp*C
  pred = b_gex * (ssum + ppi_adj.sum(-1))
  out  = relu(LN(pred)) @ W_ff

Sharding: data-parallel over batch across 8 cores (8 samples each);
weights replicated.

Performance structure (cost-model driven; the DMA device is the
roofline at ~360 GB/s with all transfers globally serialized):
  * ppi_adj, W_ff, w_gex, w_comp ride gpsimd SWDGE casting DMAs
    (f32 HBM -> bf16 SBUF) priced at OUTPUT bytes -- half the f32 DMA
    cost.  bf16 is well inside the 2e-2 relative-error gate.
  * Gene dim is tiled stride-7 interleaved: main tiles t=0..6 hold gene
    7p+t at partition p (one descriptor covers 7 contiguous HBM rows),
    tail tile holds genes 896+p.  Weight/vector gene slices become
    [t:896:7] strided APs, so gamma/beta/w_dose/w_time load as single
    natural-row descriptors.
  * ppi row-sums run on the PE: transpose-accumulate 128-wide column
    blocks into PSUM, then a ones-vector matmul.
  * LayerNorm rstd = exp(-0.5*ln(var+eps)): ln+exp share one ACT table
    set, so the kernel does exactly one 1.3us table load (primed at t=0).
    The affine+ReLU is a single ACT activation with per-partition
    scale/bias (gamma/beta).
  * Engines are strictly in-order, so program order is scheduled by
    hand: ACT runs exp before the nfT/relu chain; the pred chain
    alternates DVE/Pool tiles; the FFN accumulates per k-tile as W_ff
    chunks stream in.
"""

import numpy as np

import concourse.bass as bass
import concourse.mybir as mybir
import concourse.tile as tile
from concourse.bass_utils import run_bass_kernel_spmd
from concourse.masks import make_identity

F32 = mybir.dt.float32
F32R = mybir.dt.float32r
BF16 = mybir.dt.bfloat16
AF = mybir.ActivationFunctionType
ALU = mybir.AluOpType
AX = mybir.AxisListType

G, H, NA, FEAT, CH = 978, 128, 50, 34, 64
B, NCORES = 64, 8
BL = B // NCORES  # per-core batch
LN_EPS = 1e-5
NT_MAIN, TAIL = 7, 82
NT = NT_MAIN + 1

_DMA_ZERO_WAIT = ("InstDMACopy", "InstDMATransposeAnt", "InstTriggeredCopy")


def _split_excess_waits(nc):
    """walrus in this container accepts at most 1 inline sync-wait per
    instruction (0 for DMA).  Move excess waits onto same-engine nops
    inserted immediately before the overloaded instruction."""

    def make_nop(engine):
        bi = nc.engines[engine].nop(nofuse=True)
        ins = bi.ins
        lst = nc.cur_bb.bb.instructions
        assert lst[-1] is ins
        lst.pop()
        return ins

    for bb in nc.main_func.blocks:
        lst = bb.instructions
        i = 0
        while i < len(lst):
            ins = lst[i]
            si = getattr(ins, "sync_info", None)
            waits = list(si.on_wait) if (si and si.on_wait) else []
            limit = 0 if type(ins).__name__ in _DMA_ZERO_WAIT else 1
            if len(waits) > limit:
                keep = waits[len(waits) - limit:] if limit else []
                excess = waits[: len(waits) - limit]
                si.on_wait = keep
                pos = i
                for w in excess:
                    nop = make_nop(ins.engine)
                    nop.sync_info = mybir.SyncInfo(on_wait=[w], on_update=[])
                    lst.insert(pos, nop)
                    pos += 1
                    i += 1
            i += 1


def _gslice(ap, t):
    """Gene-slice of the last axis of a natural [*, G] AP for tile t."""
    if t < NT_MAIN:
        return ap[..., t:896:7]
    return ap[..., 896:978]


def _gn(t):
    return 128 if t < NT_MAIN else TAIL


def build_nc():
    nc = bass.Bass()

    b_gex = nc.dram_tensor("b_gex", [BL, G], F32, kind="ExternalInput")
    node_feat = nc.dram_tensor("node_feat", [BL, NA, FEAT], F32, kind="ExternalInput")
    mask = nc.dram_tensor("mask", [BL, NA], F32, kind="ExternalInput")
    adj = nc.dram_tensor("adj_matrix", [BL, NA, NA], F32, kind="ExternalInput")
    dist = nc.dram_tensor("dist_matrix", [BL, NA, NA], F32, kind="ExternalInput")
    dose = nc.dram_tensor("dose", [BL, 1], F32, kind="ExternalInput")
    time_in = nc.dram_tensor("time", [BL, 1], F32, kind="ExternalInput")
    ppi = nc.dram_tensor("ppi_adj", [G, G], F32, kind="ExternalInput")
    w_gex = nc.dram_tensor("w_gex", [G, H], F32, kind="ExternalInput")
    w_comp = nc.dram_tensor("w_comp", [G, H], F32, kind="ExternalInput")
    W1 = nc.dram_tensor("W1", [FEAT, CH], F32, kind="ExternalInput")
    W2 = nc.dram_tensor("W2", [CH, G], F32, kind="ExternalInput")
    w_dose = nc.dram_tensor("w_dose", [1, G], F32, kind="ExternalInput")
    w_time = nc.dram_tensor("w_time", [1, G], F32, kind="ExternalInput")
    ln_gamma = nc.dram_tensor("ln_gamma", [G], F32, kind="ExternalInput")
    ln_beta = nc.dram_tensor("ln_beta", [G], F32, kind="ExternalInput")
    W_ff = nc.dram_tensor("W_ff", [G, G], F32, kind="ExternalInput")

    out_pred = nc.dram_tensor("out_pred", [BL, G], F32, kind="ExternalOutput")
    # comp in gene-tile layout (bf16); kernel() reassembles with numpy.
    out_comp_m = nc.dram_tensor("out_comp_m", [128, NT_MAIN, BL], BF16,
                                kind="ExternalOutput")
    out_comp_t = nc.dram_tensor("out_comp_t", [TAIL, BL], BF16, kind="ExternalOutput")

    inv_sqrt_h = 1.0 / float(np.sqrt(H))

    with tile.TileContext(nc) as tc:
        with (
            tc.tile_pool(name="const", bufs=1) as const,
            tc.tile_pool(name="sb", bufs=1) as sb,
            tc.tile_pool(name="work", bufs=4) as work,
            tc.tile_pool(name="pacc", bufs=1, space="PSUM") as pacc,
            tc.tile_pool(name="pcyc", bufs=4, space="PSUM") as pcyc,
        ):
            ident = const.tile([128, 128], F32)
            make_identity(nc, ident[:])
            ident_bf = const.tile([128, 128], BF16)
            make_identity(nc, ident_bf[:])
            ones_col = const.tile([128, 1], F32)
            nc.vector.memset(ones_col[:], 1.0)
            ones_col_bf = const.tile([128, 1], BF16)
            nc.vector.memset(ones_col_bf[:], 1.0)
            ones_row = const.tile([1, 128], F32)
            nc.vector.memset(ones_row[:], 1.0)
            eps_t = const.tile([1, 1], F32)
            nc.vector.memset(eps_t[:], LN_EPS)
            dummy = const.tile([1, 1], F32)

            _cyc_n = [0]

            def cyc(shape, dtype=F32, name=None):
                _cyc_n[0] += 1
                return pcyc.tile(shape, dtype, tag="cyc",
                                 name=name or f"cyc{_cyc_n[0]}")

            # ===== ACT: prime the ln+exp table set, then CCE loads.
            # dist/adj issue first; exp(-dist) fires the moment dist lands.
            nc.scalar.activation(dummy[:], eps_t[:], AF.Ln)
            nc.scalar.activation(dummy[:], eps_t[:], AF.Exp)
            distT = sb.tile([NA, BL, NA], F32)
            nc.scalar.dma_start(out=distT[:], in_=dist[:, :, :].rearrange("b n m -> n b m"))
            adjT = sb.tile([NA, BL, NA], F32)
            nc.scalar.dma_start(out=adjT[:], in_=adj[:, :, :].rearrange("b n m -> n b m"))
            wmsg = sb.tile([NA, BL, NA], F32)
            nc.scalar.activation(wmsg[:], distT[:], AF.Exp, scale=-1.0)
            bg_tl = sb.tile([BL, TAIL], F32)
            nc.scalar.dma_start(out=bg_tl[:], in_=b_gex[:, 896:G])
            dt2 = sb.tile([2, BL], F32)
            nc.scalar.dma_start(out=dt2[0:1, :], in_=dose[:, :].rearrange("b o -> o b"))
            nc.scalar.dma_start(out=dt2[1:2, :], in_=time_in[:, :].rearrange("b o -> o b"))

            # ===== Pool SWDGE: casting loads (f32 -> bf16) =====
            ppi_sb = sb.tile([128, NT, G], BF16)
            ppi_r = ppi[0:896, :].rearrange("(p t) k -> p t k", p=128)
            nc.gpsimd.dma_start(out=ppi_sb[:, 0:4, :], in_=ppi_r[:, 0:4, :])
            wg_sb = sb.tile([128, NT, H], BF16)
            nc.gpsimd.dma_start(out=wg_sb[:, 0:7, :],
                                in_=w_gex[0:896, :].rearrange("(p t) h -> p t h", p=128))
            wc_sb = sb.tile([128, NT, H], BF16)
            nc.gpsimd.dma_start(out=wc_sb[:, 0:7, :],
                                in_=w_comp[0:896, :].rearrange("(p t) h -> p t h", p=128))
            nc.gpsimd.dma_start(out=ppi_sb[:, 4:7, :], in_=ppi_r[:, 4:7, :])
            nc.gpsimd.dma_start(out=ppi_sb[:TAIL, 7, :], in_=ppi[896:G, :])
            wff_sb = sb.tile([128, NT, G], BF16)
            wff_r = W_ff[0:896, :].rearrange("(p t) k -> p t k", p=128)
            nc.gpsimd.dma_start(out=wff_sb[:, 0:3, :], in_=wff_r[:, 0:3, :])
            nc.gpsimd.dma_start(out=wff_sb[:, 3:5, :], in_=wff_r[:, 3:5, :])
            nc.gpsimd.dma_start(out=wff_sb[:, 5:7, :], in_=wff_r[:, 5:7, :])
            nc.gpsimd.dma_start(out=wff_sb[:TAIL, 7, :], in_=W_ff[896:G, :])

            # ===== SP: nf, mask, W1, bgT, W2, then strided vectors =====
            nf_nat = sb.tile([100, 4, FEAT], F32)
            nc.sync.dma_start(out=nf_nat[:],
                              in_=node_feat.rearrange("b n f -> (b n) f")
                              .rearrange("(t p) f -> p t f", p=100))
            mask_nat = sb.tile([BL, NA], F32)
            nc.sync.dma_start(out=mask_nat[:], in_=mask[:, :])
            W1_sb = sb.tile([FEAT, CH], F32)
            nc.sync.dma_start(out=W1_sb[:], in_=W1[:, :])
            bgT = sb.tile([128, BL, NT_MAIN], F32)  # [p, b, t]
            nc.sync.dma_start(out=bgT[:],
                              in_=b_gex[:, 0:896].rearrange("b (p t) -> p b t", p=128))
            # W2_ext = [W2; w_dose; w_time] so comp is ONE matmul per tile
            W2_sb = sb.tile([CH + 2, G], F32)
            nc.sync.dma_start(out=W2_sb[0:CH, :], in_=W2[:, :])
            nc.sync.dma_start(out=W2_sb[CH:CH + 1, :], in_=w_dose[:, :])
            nc.sync.dma_start(out=W2_sb[CH + 1:CH + 2, :], in_=w_time[:, :])
            # gamma/beta in per-tile per-partition layout
            wgt_f = sb.tile([TAIL, H], F32)
            nc.sync.dma_start(out=wgt_f[:], in_=w_gex[896:G, :])
            wct_f = sb.tile([TAIL, H], F32)
            nc.sync.dma_start(out=wct_f[:], in_=w_comp[896:G, :])
            gam_sb = sb.tile([128, NT], F32)
            nc.sync.dma_start(out=gam_sb[:, 0:7],
                              in_=ln_gamma[0:896].rearrange("(p t) -> p t", p=128))
            nc.sync.dma_start(out=gam_sb[:TAIL, 7:8],
                              in_=ln_gamma[896:G].rearrange("(p o) -> p o", o=1))
            bet_sb = sb.tile([128, NT], F32)
            nc.sync.dma_start(out=bet_sb[:, 0:7],
                              in_=ln_beta[0:896].rearrange("(p t) -> p t", p=128))
            nc.sync.dma_start(out=bet_sb[:TAIL, 7:8],
                              in_=ln_beta[896:G].rearrange("(p o) -> p o", o=1))


            # ===== packed PSUM accumulators =====
            u_ps = pacc.tile([H, BL], F32, tag="u")
            prs_ps = pacc.tile([128, NT], F32, tag="prs")
            cT_ps = pacc.tile([128, NT, BL], F32, tag="ct")
            st_ps = pacc.tile([1, 2 * BL], F32, tag="stats")  # [x | x^2]

            # ===== CCE =====
            # wmsg = adj * exp(-dist)  (exp emitted above, right after loads)
            nc.vector.tensor_mul(wmsg[:], wmsg[:], adjT[:])

            nfT_ps = cyc([FEAT, BL * NA])
            for j in range(4):
                nc.tensor.transpose(nfT_ps[:, j * 100:(j + 1) * 100],
                                    nf_nat[:, j, :], ident[:100, :100])
            nfT = sb.tile([FEAT, BL * NA], F32)
            nc.vector.tensor_copy(nfT[:], nfT_ps[:])

            # bf16 b_gex copies + w tails early in the DVE queue
            bgT_bf = sb.tile([128, BL, NT_MAIN], BF16)
            nc.vector.tensor_copy(bgT_bf[:].rearrange("p b t -> p (b t)"),
                                  bgT[:].rearrange("p b t -> p (b t)"))
            nc.vector.tensor_copy(wg_sb[:TAIL, 7, :], wgt_f[:])
            nc.vector.tensor_copy(wc_sb[:TAIL, 7, :], wct_f[:])

            mT_ps = cyc([NA, BL])
            nc.tensor.transpose(mT_ps[:], mask_nat[:], ident[:BL, :BL])
            maskT = sb.tile([NA, BL], F32)
            nc.vector.tensor_copy(maskT[:], mT_ps[:])

            # h2[b] = relu(nf_b @ W1) in [n, d] layout (per-sample matmuls)
            h2_ps = cyc([NA, BL, CH])
            for b in range(BL):
                nc.tensor.matmul(h2_ps[:, b, :], nfT[:, b * NA:(b + 1) * NA],
                                 W1_sb[:], start=True, stop=True)
            h2 = sb.tile([NA, BL, CH], F32)
            nc.scalar.activation(h2[:].rearrange("n b d -> n (b d)"),
                                 h2_ps[:].rearrange("n b d -> n (b d)"), AF.Relu)

            # gT[m, b] = sum_n mask[b,n] wmsg[n,b,m]  (per-sample PE matmuls)
            gT_ps = cyc([NA, BL])
            for b in range(BL):
                nc.tensor.matmul(gT_ps[:, b:b + 1], wmsg[:, b, :],
                                 maskT[:, b:b + 1], start=True, stop=True)
            gT_sb = sb.tile([NA, BL], F32)
            nc.vector.tensor_copy(gT_sb[:], gT_ps[:])

            # pooled[d, b] = sum_m h2[m, b, d] * gT[m, b]
            pool_ps = cyc([CH, BL])
            for b in range(BL):
                nc.tensor.matmul(pool_ps[:, b:b + 1], h2[:, b, :],
                                 gT_sb[:, b:b + 1], start=True, stop=True)

            ms_ps = cyc([1, BL])
            nc.tensor.matmul(ms_ps[:], ones_col[:NA, :], maskT[:], start=True, stop=True)
            ms_sb = sb.tile([1, BL], F32)
            nc.vector.tensor_scalar_max(ms_sb[:], ms_ps[:], 1.0)
            rms = sb.tile([1, BL], F32)
            nc.vector.reciprocal(rms[:], ms_sb[:])
            rb_ps = cyc([CH, BL])
            nc.tensor.matmul(rb_ps[:], ones_row[:1, :CH], rms[:], start=True, stop=True)
            rb_sb = sb.tile([CH, BL], F32)
            nc.vector.tensor_copy(rb_sb[:], rb_ps[:])
            pooled_ext = sb.tile([CH + 2, BL], F32)
            nc.vector.tensor_mul(pooled_ext[0:CH, :], pool_ps[:], rb_sb[:])
            nc.vector.tensor_copy(pooled_ext[CH:CH + 2, :], dt2[:])

            # b_gex tail -> [TAIL, BL] (bf16)
            bgt_ps = cyc([TAIL, BL])
            nc.tensor.transpose(bgt_ps[:], bg_tl[:], ident[:BL, :BL])
            bgT_tbf = sb.tile([TAIL, BL], BF16)
            nc.vector.tensor_copy(bgT_tbf[:], bgt_ps[:])

            # ===== ppi row-sums on the PE (tiles 0..3 from chunk A) =====
            # narrow 82-wide block mid-group so start/stop cover full region
            KBLK = [(0, 128), (896, TAIL)] + [(c * 128, 128) for c in range(1, 7)]
            prs_sb = sb.tile([128, NT], F32)

            def prs_tile(t):
                gn = _gn(t)
                # transpose via plain matmul with identity rhs: f32 PSUM
                # accumulate works on HW (bf16 PSUM does not)
                S_ps = cyc([128, 128], name=f"S{t}")
                for c, (k0, kw) in enumerate(KBLK):
                    nc.tensor.matmul(S_ps[:kw, :gn], ppi_sb[:gn, t, k0:k0 + kw],
                                     ident_bf[:gn, :gn],
                                     start=(c == 0), stop=(c == len(KBLK) - 1))
                S_sb = work.tile([128, 128], BF16, tag="S_sb")
                if t % 2 == 0:
                    nc.scalar.copy(S_sb[:, :gn], S_ps[:, :gn])
                else:
                    nc.vector.tensor_copy(S_sb[:, :gn], S_ps[:, :gn])
                nc.tensor.matmul(prs_ps[:gn, t:t + 1], S_sb[:, :gn],
                                 ones_col_bf[:128, :], start=True, stop=True)
                if t % 2 == 0:
                    nc.vector.tensor_copy(prs_sb[:gn, t:t + 1], prs_ps[:gn, t:t + 1])
                else:
                    nc.scalar.copy(prs_sb[:gn, t:t + 1], prs_ps[:gn, t:t + 1])

            # ===== u: b_gex half starts as soon as bgT/wg land =====
            for t in range(NT):
                bg_rhs = bgT_bf[:, :, t] if t < NT_MAIN else bgT_tbf[:, :]
                nc.tensor.matmul(u_ps[:], wg_sb[:_gn(t), t, :], bg_rhs,
                                 start=(t == 0), stop=False)

            # ===== wgcT: transposed w_gex/w_comp tiles (for A/C) =====
            wgcT = []
            for pr in range(4):
                t0, t1n = 2 * pr, 2 * pr + 1
                gn1 = _gn(t1n)
                wgc_ps = cyc([128, 4, 128], name=f"wgc{pr}")
                for s, (tt, gg) in enumerate(((t0, 128), (t0, 128),
                                              (t1n, gn1), (t1n, gn1))):
                    src = wg_sb if s % 2 == 0 else wc_sb
                    nc.tensor.matmul(wgc_ps[:, s, :gg], src[:gg, tt, :],
                                     ident_bf[:gg, :gg],
                                     start=True, stop=True)
                wt = work.tile([H, 4, 128], BF16, tag="wgcT", name=f"wgcT{pr}")
                if gn1 == 128:
                    nc.scalar.copy(wt[:].rearrange("p s h -> p (s h)"),
                                   wgc_ps[:].rearrange("p s h -> p (s h)"))
                else:
                    nc.scalar.copy(wt[:, 0:2, :].rearrange("p s h -> p (s h)"),
                                   wgc_ps[:, 0:2, :].rearrange("p s h -> p (s h)"))
                    nc.scalar.copy(wt[:, 2:4, :gn1], wgc_ps[:, 2:4, :gn1])
                wgcT.append(wt)

            # ppi row-sums for chunk A tiles (PE packs into the idle window)
            for t in range(4):
                prs_tile(t)

            # ===== comp (gene-tiled): one W2_ext matmul per tile =====
            for t in range(NT):
                nc.tensor.matmul(cT_ps[:_gn(t), t, :], _gslice(W2_sb[:], t),
                                 pooled_ext[:], start=True, stop=True)
            compT = sb.tile([128, NT, BL], BF16)
            nc.scalar.copy(compT[:, 0:7, :].rearrange("p t b -> p (t b)"),
                           cT_ps[:, 0:7, :].rearrange("p t b -> p (t b)"))
            nc.vector.tensor_copy(compT[:TAIL, 7, :], cT_ps[:TAIL, 7, :])
            nc.sync.dma_start(out=out_comp_m[:, :, :], in_=compT[:, 0:7, :])
            nc.sync.dma_start(out=out_comp_t[:, :], in_=compT[:TAIL, 7, :])

            # ===== u: comp half =====
            for t in range(NT):
                nc.tensor.matmul(u_ps[:], wc_sb[:_gn(t), t, :], compT[:_gn(t), t, :],
                                 start=False, stop=(t == NT - 1))
            u_sb = sb.tile([H, BL], BF16)
            nc.scalar.activation(u_sb[:], u_ps[:], AF.Copy, scale=inv_sqrt_h)

            # ppi row-sums for the remaining tiles (chunks B / tail)
            for t in range(4, NT):
                prs_tile(t)

            # ===== A/C, pred, LN stats =====
            # predT packs [pred | pred^2] per tile so ONE matmul accumulates
            # both LN statistics (a PSUM bank allows one open group at a time)
            predT = sb.tile([128, NT, 2, BL], F32)

            def ac_tile(t):
                gn = _gn(t)
                so = (t % 2) * 2
                eng = nc.vector if t % 2 == 0 else nc.gpsimd
                ac_ps = cyc([128, 2, BL], name=f"ac{t}")
                nc.tensor.matmul(ac_ps[:gn, 0, :], wgcT[t // 2][:, so, :gn], u_sb[:],
                                 start=True, stop=True)
                nc.tensor.matmul(ac_ps[:gn, 1, :], wgcT[t // 2][:, so + 1, :gn],
                                 u_sb[:], start=True, stop=True)

                bg_t = bgT_bf[:, :, t] if t < NT_MAIN else bgT_tbf[:, :]
                # PSUM-reading ops must stay off gpsimd (HW restriction)
                t1 = work.tile([128, BL], F32, tag="t1")
                nc.vector.tensor_mul(t1[:gn, :], bg_t[:gn], ac_ps[:gn, 0, :])
                t2 = work.tile([128, BL], F32, tag="t2")
                nc.vector.tensor_mul(t2[:gn, :], compT[:gn, t, :], ac_ps[:gn, 1, :])
                nc.vector.tensor_add(t1[:gn, :], t1[:gn, :], t2[:gn, :])
                nc.vector.scalar_tensor_tensor(predT[:gn, t, 0, :], t1[:gn, :],
                                               prs_sb[:gn, t:t + 1], bg_t[:gn],
                                               op0=ALU.add, op1=ALU.mult)
                nc.gpsimd.tensor_mul(predT[:gn, t, 1, :], predT[:gn, t, 0, :],
                                     predT[:gn, t, 0, :])
                nc.tensor.matmul(st_ps[:, :],
                                 ones_col[:gn, :],
                                 predT[:gn, t, :, :].rearrange("p s b -> p (s b)"),
                                 start=(t == 0), stop=(t == NT - 1))

            for t in range(NT):
                ac_tile(t)

            # ===== LayerNorm (rstd via ln+exp: no extra ACT table) =====
            mu = sb.tile([1, BL], F32)
            nc.vector.tensor_scalar_mul(mu[:], st_ps[:, 0:BL], 1.0 / G)
            ex2 = sb.tile([1, BL], F32)
            nc.vector.tensor_scalar_mul(ex2[:], st_ps[:, BL:2 * BL], 1.0 / G)
            mu2 = sb.tile([1, BL], F32)
            nc.vector.tensor_mul(mu2[:], mu[:], mu[:])
            var = sb.tile([1, BL], F32)
            nc.vector.tensor_sub(var[:], ex2[:], mu2[:])
            lv = sb.tile([1, BL], F32)
            nc.scalar.activation(lv[:], var[:], AF.Ln, bias=eps_t[:1, 0:1])
            rstd = sb.tile([1, BL], F32)
            nc.scalar.activation(rstd[:], lv[:], AF.Exp, scale=-0.5)
            mr_ps = cyc([128, 2, BL], name="mr_bc")
            nc.tensor.matmul(mr_ps[:, 0, :], ones_row[:], mu[:], start=True, stop=True)
            nc.tensor.matmul(mr_ps[:, 1, :], ones_row[:], rstd[:], start=True, stop=True)
            rstd_sb = sb.tile([128, BL], F32)
            nc.scalar.copy(rstd_sb[:], mr_ps[:, 1, :])
            # mrs = mu * rstd (broadcast tiles)
            mrs_sb = sb.tile([128, BL], F32)
            nc.vector.tensor_mul(mrs_sb[:], mr_ps[:, 0, :], rstd_sb[:])

            # xn = relu((pred*rstd - mu*rstd)*gam + bet); affine+relu fused
            # into one ACT activation with per-partition scale/bias
            xn = sb.tile([128, NT, BL], BF16)
            for t in range(NT):
                gn = _gn(t)
                eng = nc.gpsimd
                xm = work.tile([128, BL], F32, tag="xm")
                eng.tensor_mul(xm[:gn, :], predT[:gn, t, 0, :], rstd_sb[:gn, :])
                eng.tensor_sub(xm[:gn, :], xm[:gn, :], mrs_sb[:gn, :])
                nc.scalar.activation(xn[:gn, t, :], xm[:gn, :], AF.Relu,
                                     scale=gam_sb[:gn, t:t + 1],
                                     bias=bet_sb[:gn, t:t + 1])

            # ===== FFN (bf16) =====
            NSPLIT = [(0, 512), (512, G - 512)]
            o_ps = [pcyc.tile([BL, n], F32, tag="cyc", name=f"o_ps{i}")
                    for i, (s, n) in enumerate(NSPLIT)]
            for t in range(NT):
                gn = _gn(t)
                for i, (ns, nn) in enumerate(NSPLIT):
                    nc.tensor.matmul(o_ps[i][:], xn[:gn, t, :],
                                     wff_sb[:gn, t, ns:ns + nn],
                                     start=(t == 0), stop=(t == NT - 1))
            pred_out = sb.tile([BL, G], F32)
            nc.scalar.copy(pred_out[:, 0:512], o_ps[0][:])
            nc.vector.tensor_copy(pred_out[:, 512:G], o_ps[1][:])
            nc.sync.dma_start(out=out_pred[:, :], in_=pred_out[:])

    _split_excess_waits(nc)
    return nc


_PER_SAMPLE = ("b_gex", "node_feat", "mask", "adj_matrix", "dist_matrix", "dose", "time")


def kernel(**inputs):
    inputs = {k: np.ascontiguousarray(np.asarray(v, dtype=np.float32))
              for k, v in inputs.items()}
    nc = build_nc()
    in_maps = []
    for c in range(NCORES):
        m = {}
        for k, v in inputs.items():
            if k in _PER_SAMPLE:
                m[k] = np.ascontiguousarray(v[c * BL:(c + 1) * BL])
            else:
                m[k] = v
        in_maps.append(m)
    r = run_bass_kernel_spmd(nc, in_maps, list(range(NCORES)))
    pred = np.concatenate([r.results[c]["out_pred"] for c in range(NCORES)], axis=0)
    comps = []
    for c in range(NCORES):
        cm = np.asarray(r.results[c]["out_comp_m"], np.float32)  # [128, 7, BL]
        ct = np.asarray(r.results[c]["out_comp_t"], np.float32)  # [TAIL, BL]
        full = np.empty((BL, G), np.float32)
        full[:, 0:896] = cm.transpose(2, 0, 1).reshape(BL, 896)
        full[:, 896:G] = ct.T
        comps.append(full)
    comp = np.concatenate(comps, axis=0)
    return pred, comp


# revision 34
# speedup vs baseline: 1.0179x; 1.0179x over previous
"""Trainium2 Bass kernel for nn_CSG2A_net (gnn_message_passing).

Math (algebraically identical to the reference, never materializes the
[B,G,G] score tensor):
  CCE:  h = relu(node_feat @ W1); w = adj*exp(-dist)
        g[b,m] = sum_n mask[b,n] * w[b,n,m]
        pooled[b,d] = (sum_m g[b,m] h[b,m,d]) / clip(sum_n mask[b,n], 1)
        comp = pooled @ W2 + dose @ w_dose + time @ w_time
  score.sum(-1)[b,g] = q[b,g,:] . (sum_k q[b,k,:]) / sqrt(H)
    with q[b,g,:] = b_gex[b,g] w_gex[g,:] + comp[b,g] w_comp[g,:]
    so  u = (b_gex @ w_gex + comp @ w_comp) / sqrt(H)   [H,B]
        A = w_gex @ u ; C = w_comp @ u                  [G,B]
        ssum = b_gex*A + comp*C
  pred = b_gex * (ssum + ppi_adj.sum(-1))
  out  = relu(LN(pred)) @ W_ff

Sharding: data-parallel over batch across 8 cores (8 samples each);
weights replicated.

Performance structure (cost-model driven; the DMA device is the
roofline at ~360 GB/s with all transfers globally serialized):
  * ppi_adj, W_ff, w_gex, w_comp ride gpsimd SWDGE casting DMAs
    (f32 HBM -> bf16 SBUF) priced at OUTPUT bytes -- half the f32 DMA
    cost.  bf16 is well inside the 2e-2 relative-error gate.
  * Gene dim is tiled stride-7 interleaved: main tiles t=0..6 hold gene
    7p+t at partition p (one descriptor covers 7 contiguous HBM rows),
    tail tile holds genes 896+p.  Weight/vector gene slices become
    [t:896:7] strided APs, so gamma/beta/w_dose/w_time load as single
    natural-row descriptors.
  * ppi row-sums run on the PE: transpose-accumulate 128-wide column
    blocks into PSUM, then a ones-vector matmul.
  * LayerNorm rstd = exp(-0.5*ln(var+eps)): ln+exp share one ACT table
    set, so the kernel does exactly one 1.3us table load (primed at t=0).
    The affine+ReLU is a single ACT activation with per-partition
    scale/bias (gamma/beta).
  * Engines are strictly in-order, so program order is scheduled by
    hand: ACT runs exp before the nfT/relu chain; the pred chain
    alternates DVE/Pool tiles; the FFN accumulates per k-tile as W_ff
    chunks stream in.
"""

import numpy as np

import concourse.bass as bass
import concourse.mybir as mybir
import concourse.tile as tile
from concourse.bass_utils import run_bass_kernel_spmd
from concourse.masks import make_identity

F32 = mybir.dt.float32
F32R = mybir.dt.float32r
BF16 = mybir.dt.bfloat16
AF = mybir.ActivationFunctionType
ALU = mybir.AluOpType
AX = mybir.AxisListType

G, H, NA, FEAT, CH = 978, 128, 50, 34, 64
B, NCORES = 64, 8
BL = B // NCORES  # per-core batch
LN_EPS = 1e-5
NT_MAIN, TAIL = 7, 82
NT = NT_MAIN + 1

_DMA_ZERO_WAIT = ("InstDMACopy", "InstDMATransposeAnt", "InstTriggeredCopy")


def _split_excess_waits(nc):
    """walrus in this container accepts at most 1 inline sync-wait per
    instruction (0 for DMA).  Move excess waits onto same-engine nops
    inserted immediately before the overloaded instruction."""

    def make_nop(engine):
        bi = nc.engines[engine].nop(nofuse=True)
        ins = bi.ins
        lst = nc.cur_bb.bb.instructions
        assert lst[-1] is ins
        lst.pop()
        return ins

    for bb in nc.main_func.blocks:
        lst = bb.instructions
        i = 0
        while i < len(lst):
            ins = lst[i]
            si = getattr(ins, "sync_info", None)
            waits = list(si.on_wait) if (si and si.on_wait) else []
            limit = 0 if type(ins).__name__ in _DMA_ZERO_WAIT else 1
            if len(waits) > limit:
                keep = waits[len(waits) - limit:] if limit else []
                excess = waits[: len(waits) - limit]
                si.on_wait = keep
                pos = i
                for w in excess:
                    nop = make_nop(ins.engine)
                    nop.sync_info = mybir.SyncInfo(on_wait=[w], on_update=[])
                    lst.insert(pos, nop)
                    pos += 1
                    i += 1
            i += 1


def _gslice(ap, t):
    """Gene-slice of the last axis of a natural [*, G] AP for tile t."""
    if t < NT_MAIN:
        return ap[..., t:896:7]
    return ap[..., 896:978]


def _gn(t):
    return 128 if t < NT_MAIN else TAIL


def build_nc():
    nc = bass.Bass()

    b_gex = nc.dram_tensor("b_gex", [BL, G], F32, kind="ExternalInput")
    node_feat = nc.dram_tensor("node_feat", [BL, NA, FEAT], F32, kind="ExternalInput")
    mask = nc.dram_tensor("mask", [BL, NA], F32, kind="ExternalInput")
    adj = nc.dram_tensor("adj_matrix", [BL, NA, NA], F32, kind="ExternalInput")
    dist = nc.dram_tensor("dist_matrix", [BL, NA, NA], F32, kind="ExternalInput")
    dose = nc.dram_tensor("dose", [BL, 1], F32, kind="ExternalInput")
    time_in = nc.dram_tensor("time", [BL, 1], F32, kind="ExternalInput")
    ppi = nc.dram_tensor("ppi_adj", [G, G], F32, kind="ExternalInput")
    w_gex = nc.dram_tensor("w_gex", [G, H], F32, kind="ExternalInput")
    w_comp = nc.dram_tensor("w_comp", [G, H], F32, kind="ExternalInput")
    W1 = nc.dram_tensor("W1", [FEAT, CH], F32, kind="ExternalInput")
    W2 = nc.dram_tensor("W2", [CH, G], F32, kind="ExternalInput")
    w_dose = nc.dram_tensor("w_dose", [1, G], F32, kind="ExternalInput")
    w_time = nc.dram_tensor("w_time", [1, G], F32, kind="ExternalInput")
    ln_gamma = nc.dram_tensor("ln_gamma", [G], F32, kind="ExternalInput")
    ln_beta = nc.dram_tensor("ln_beta", [G], F32, kind="ExternalInput")
    W_ff = nc.dram_tensor("W_ff", [G, G], F32, kind="ExternalInput")

    out_pred = nc.dram_tensor("out_pred", [BL, G], F32, kind="ExternalOutput")
    # comp in gene-tile layout (bf16); kernel() reassembles with numpy.
    out_comp_m = nc.dram_tensor("out_comp_m", [128, NT_MAIN, BL], BF16,
                                kind="ExternalOutput")
    out_comp_t = nc.dram_tensor("out_comp_t", [TAIL, BL], BF16, kind="ExternalOutput")

    inv_sqrt_h = 1.0 / float(np.sqrt(H))

    with tile.TileContext(nc) as tc:
        with (
            tc.tile_pool(name="const", bufs=1) as const,
            tc.tile_pool(name="sb", bufs=1) as sb,
            tc.tile_pool(name="work", bufs=4) as work,
            tc.tile_pool(name="pacc", bufs=1, space="PSUM") as pacc,
            tc.tile_pool(name="pcyc", bufs=4, space="PSUM") as pcyc,
        ):
            ident = const.tile([128, 128], F32)
            make_identity(nc, ident[:])
            ident_bf = const.tile([128, 128], BF16)
            make_identity(nc, ident_bf[:])
            ones_col = const.tile([128, 1], F32)
            nc.vector.memset(ones_col[:], 1.0)
            ones_col_bf = const.tile([128, 1], BF16)
            nc.vector.memset(ones_col_bf[:], 1.0)
            ones_row = const.tile([1, 128], F32)
            nc.vector.memset(ones_row[:], 1.0)
            eps_t = const.tile([1, 1], F32)
            nc.vector.memset(eps_t[:], LN_EPS)
            dummy = const.tile([1, 1], F32)

            _cyc_n = [0]

            def cyc(shape, dtype=F32, name=None):
                _cyc_n[0] += 1
                return pcyc.tile(shape, dtype, tag="cyc",
                                 name=name or f"cyc{_cyc_n[0]}")

            # ===== ACT: prime the ln+exp table set, then CCE loads.
            # dist/adj issue first; exp(-dist) fires the moment dist lands.
            nc.scalar.activation(dummy[:], eps_t[:], AF.Ln)
            nc.scalar.activation(dummy[:], eps_t[:], AF.Exp)
            distT = sb.tile([NA, BL, NA], F32)
            nc.scalar.dma_start(out=distT[:], in_=dist[:, :, :].rearrange("b n m -> n b m"))
            adjT = sb.tile([NA, BL, NA], F32)
            nc.scalar.dma_start(out=adjT[:], in_=adj[:, :, :].rearrange("b n m -> n b m"))
            wmsg = sb.tile([NA, BL, NA], F32)
            nc.scalar.activation(wmsg[:], distT[:], AF.Exp, scale=-1.0)
            bg_tl = sb.tile([BL, TAIL], F32)
            nc.scalar.dma_start(out=bg_tl[:], in_=b_gex[:, 896:G])
            dt2 = sb.tile([2, BL], F32)
            nc.scalar.dma_start(out=dt2[0:1, :], in_=dose[:, :].rearrange("b o -> o b"))
            nc.scalar.dma_start(out=dt2[1:2, :], in_=time_in[:, :].rearrange("b o -> o b"))

            # ===== Pool SWDGE: casting loads (f32 -> bf16) =====
            ppi_sb = sb.tile([128, NT, G], BF16)
            ppi_r = ppi[0:896, :].rearrange("(p t) k -> p t k", p=128)
            nc.gpsimd.dma_start(out=ppi_sb[:, 0:4, :], in_=ppi_r[:, 0:4, :])
            wg_sb = sb.tile([128, NT, H], BF16)
            nc.gpsimd.dma_start(out=wg_sb[:, 0:7, :],
                                in_=w_gex[0:896, :].rearrange("(p t) h -> p t h", p=128))
            wc_sb = sb.tile([128, NT, H], BF16)
            nc.gpsimd.dma_start(out=wc_sb[:, 0:7, :],
                                in_=w_comp[0:896, :].rearrange("(p t) h -> p t h", p=128))
            nc.gpsimd.dma_start(out=ppi_sb[:, 4:7, :], in_=ppi_r[:, 4:7, :])
            nc.gpsimd.dma_start(out=ppi_sb[:TAIL, 7, :], in_=ppi[896:G, :])
            wff_sb = sb.tile([128, NT, G], BF16)
            wff_r = W_ff[0:896, :].rearrange("(p t) k -> p t k", p=128)
            nc.gpsimd.dma_start(out=wff_sb[:, 0:3, :], in_=wff_r[:, 0:3, :])
            nc.gpsimd.dma_start(out=wff_sb[:, 3:5, :], in_=wff_r[:, 3:5, :])
            nc.gpsimd.dma_start(out=wff_sb[:, 5:7, :], in_=wff_r[:, 5:7, :])
            nc.gpsimd.dma_start(out=wff_sb[:TAIL, 7, :], in_=W_ff[896:G, :])

            # ===== SP: nf, mask, W1, bgT, W2, then strided vectors =====
            nf_nat = sb.tile([100, 4, FEAT], F32)
            nc.sync.dma_start(out=nf_nat[:],
                              in_=node_feat.rearrange("b n f -> (b n) f")
                              .rearrange("(t p) f -> p t f", p=100))
            mask_nat = sb.tile([BL, NA], F32)
            nc.sync.dma_start(out=mask_nat[:], in_=mask[:, :])
            W1_sb = sb.tile([FEAT, CH], F32)
            nc.sync.dma_start(out=W1_sb[:], in_=W1[:, :])
            bgT = sb.tile([128, BL, NT_MAIN], F32)  # [p, b, t]
            nc.sync.dma_start(out=bgT[:],
                              in_=b_gex[:, 0:896].rearrange("b (p t) -> p b t", p=128))
            # W2_ext = [W2; w_dose; w_time] so comp is ONE matmul per tile
            W2_sb = sb.tile([CH + 2, G], F32)
            nc.sync.dma_start(out=W2_sb[0:CH, :], in_=W2[:, :])
            nc.sync.dma_start(out=W2_sb[CH:CH + 1, :], in_=w_dose[:, :])
            nc.sync.dma_start(out=W2_sb[CH + 1:CH + 2, :], in_=w_time[:, :])
            # gamma/beta in per-tile per-partition layout
            wgt_f = sb.tile([TAIL, H], F32)
            nc.sync.dma_start(out=wgt_f[:], in_=w_gex[896:G, :])
            wct_f = sb.tile([TAIL, H], F32)
            nc.sync.dma_start(out=wct_f[:], in_=w_comp[896:G, :])
            gam_sb = sb.tile([128, NT], F32)
            nc.sync.dma_start(out=gam_sb[:, 0:7],
                              in_=ln_gamma[0:896].rearrange("(p t) -> p t", p=128))
            nc.sync.dma_start(out=gam_sb[:TAIL, 7:8],
                              in_=ln_gamma[896:G].rearrange("(p o) -> p o", o=1))
            bet_sb = sb.tile([128, NT], F32)
            nc.sync.dma_start(out=bet_sb[:, 0:7],
                              in_=ln_beta[0:896].rearrange("(p t) -> p t", p=128))
            nc.sync.dma_start(out=bet_sb[:TAIL, 7:8],
                              in_=ln_beta[896:G].rearrange("(p o) -> p o", o=1))


            # ===== packed PSUM accumulators =====
            u_ps = pacc.tile([H, BL], F32, tag="u")
            prs_ps = pacc.tile([128, NT], F32, tag="prs")
            cT_ps = pacc.tile([128, NT, BL], F32, tag="ct")
            st_ps = pacc.tile([1, 2 * BL], F32, tag="stats")  # [x | x^2]

            # ===== CCE =====
            # wmsg = adj * exp(-dist)  (exp emitted above, right after loads)
            nc.vector.tensor_mul(wmsg[:], wmsg[:], adjT[:])

            nfT_ps = cyc([FEAT, BL * NA])
            for j in range(4):
                nc.tensor.transpose(nfT_ps[:, j * 100:(j + 1) * 100],
                                    nf_nat[:, j, :], ident[:100, :100])
            nfT = sb.tile([FEAT, BL * NA], F32)
            nc.vector.tensor_copy(nfT[:], nfT_ps[:])

            # bf16 b_gex copies + w tails early in the DVE queue
            bgT_bf = sb.tile([128, BL, NT_MAIN], BF16)
            nc.vector.tensor_copy(bgT_bf[:].rearrange("p b t -> p (b t)"),
                                  bgT[:].rearrange("p b t -> p (b t)"))
            nc.vector.tensor_copy(wg_sb[:TAIL, 7, :], wgt_f[:])
            nc.vector.tensor_copy(wc_sb[:TAIL, 7, :], wct_f[:])

            mT_ps = cyc([NA, BL])
            nc.tensor.transpose(mT_ps[:], mask_nat[:], ident[:BL, :BL])
            maskT = sb.tile([NA, BL], F32)
            nc.vector.tensor_copy(maskT[:], mT_ps[:])

            # h2[b] = relu(nf_b @ W1) in [n, d] layout (per-sample matmuls)
            h2_ps = cyc([NA, BL, CH])
            for b in range(BL):
                nc.tensor.matmul(h2_ps[:, b, :], nfT[:, b * NA:(b + 1) * NA],
                                 W1_sb[:], start=True, stop=True)
            h2 = sb.tile([NA, BL, CH], F32)
            nc.scalar.activation(h2[:].rearrange("n b d -> n (b d)"),
                                 h2_ps[:].rearrange("n b d -> n (b d)"), AF.Relu)

            # gT[m, b] = sum_n mask[b,n] wmsg[n,b,m]  (per-sample PE matmuls)
            gT_ps = cyc([NA, BL])
            for b in range(BL):
                nc.tensor.matmul(gT_ps[:, b:b + 1], wmsg[:, b, :],
                                 maskT[:, b:b + 1], start=True, stop=True)
            gT_sb = sb.tile([NA, BL], F32)
            nc.vector.tensor_copy(gT_sb[:], gT_ps[:])

            # pooled[d, b] = sum_m h2[m, b, d] * gT[m, b]
            pool_ps = cyc([CH, BL])
            for b in range(BL):
                nc.tensor.matmul(pool_ps[:, b:b + 1], h2[:, b, :],
                                 gT_sb[:, b:b + 1], start=True, stop=True)

            ms_ps = cyc([1, BL])
            nc.tensor.matmul(ms_ps[:], ones_col[:NA, :], maskT[:], start=True, stop=True)
            ms_sb = sb.tile([1, BL], F32)
            nc.vector.tensor_scalar_max(ms_sb[:], ms_ps[:], 1.0)
            rms = sb.tile([1, BL], F32)
            nc.vector.reciprocal(rms[:], ms_sb[:])
            rb_ps = cyc([CH, BL])
            nc.tensor.matmul(rb_ps[:], ones_row[:1, :CH], rms[:], start=True, stop=True)
            rb_sb = sb.tile([CH, BL], F32)
            nc.vector.tensor_copy(rb_sb[:], rb_ps[:])
            pooled_ext = sb.tile([CH + 2, BL], F32)
            nc.vector.tensor_mul(pooled_ext[0:CH, :], pool_ps[:], rb_sb[:])
            nc.vector.tensor_copy(pooled_ext[CH:CH + 2, :], dt2[:])

            # b_gex tail -> [TAIL, BL] (bf16)
            bgt_ps = cyc([TAIL, BL])
            nc.tensor.transpose(bgt_ps[:], bg_tl[:], ident[:BL, :BL])
            bgT_tbf = sb.tile([TAIL, BL], BF16)
            nc.vector.tensor_copy(bgT_tbf[:], bgt_ps[:])

            # ===== ppi row-sums on the PE (tiles 0..3 from chunk A) =====
            # narrow 82-wide block mid-group so start/stop cover full region
            KBLK = [(0, 128), (896, TAIL)] + [(c * 128, 128) for c in range(1, 7)]
            prs_sb = sb.tile([128, NT], F32)

            def prs_tile(t):
                gn = _gn(t)
                # transpose via plain matmul with identity rhs: f32 PSUM
                # accumulate works on HW (bf16 PSUM does not)
                S_ps = cyc([128, 128], name=f"S{t}")
                for c, (k0, kw) in enumerate(KBLK):
                    nc.tensor.matmul(S_ps[:kw, :gn], ppi_sb[:gn, t, k0:k0 + kw],
                                     ident_bf[:gn, :gn],
                                     start=(c == 0), stop=(c == len(KBLK) - 1))
                S_sb = work.tile([128, 128], BF16, tag="S_sb")
                if t % 2 == 0:
                    nc.vector.tensor_copy(S_sb[:, :gn], S_ps[:, :gn])
                else:
                    nc.scalar.copy(S_sb[:, :gn], S_ps[:, :gn])
                nc.tensor.matmul(prs_ps[:gn, t:t + 1], S_sb[:, :gn],
                                 ones_col_bf[:128, :], start=True, stop=True)
                if t % 2 == 0:
                    nc.vector.tensor_copy(prs_sb[:gn, t:t + 1], prs_ps[:gn, t:t + 1])
                else:
                    nc.scalar.copy(prs_sb[:gn, t:t + 1], prs_ps[:gn, t:t + 1])

            # ===== u: b_gex half starts as soon as bgT/wg land =====
            for t in range(NT):
                bg_rhs = bgT_bf[:, :, t] if t < NT_MAIN else bgT_tbf[:, :]
                nc.tensor.matmul(u_ps[:], wg_sb[:_gn(t), t, :], bg_rhs,
                                 start=(t == 0), stop=False)

            # ===== wgcT: transposed w_gex/w_comp tiles (for A/C) =====
            wgcT = []
            for pr in range(4):
                t0, t1n = 2 * pr, 2 * pr + 1
                gn1 = _gn(t1n)
                wgc_ps = cyc([128, 4, 128], name=f"wgc{pr}")
                for s, (tt, gg) in enumerate(((t0, 128), (t0, 128),
                                              (t1n, gn1), (t1n, gn1))):
                    src = wg_sb if s % 2 == 0 else wc_sb
                    nc.tensor.matmul(wgc_ps[:, s, :gg], src[:gg, tt, :],
                                     ident_bf[:gg, :gg],
                                     start=True, stop=True)
                wt = work.tile([H, 4, 128], BF16, tag="wgcT", name=f"wgcT{pr}")
                if gn1 == 128:
                    nc.scalar.copy(wt[:].rearrange("p s h -> p (s h)"),
                                   wgc_ps[:].rearrange("p s h -> p (s h)"))
                else:
                    nc.scalar.copy(wt[:, 0:2, :].rearrange("p s h -> p (s h)"),
                                   wgc_ps[:, 0:2, :].rearrange("p s h -> p (s h)"))
                    nc.scalar.copy(wt[:, 2:4, :gn1], wgc_ps[:, 2:4, :gn1])
                wgcT.append(wt)

            # ppi row-sums for chunk A tiles (PE packs into the idle window)
            for t in range(4):
                prs_tile(t)

            # ===== comp (gene-tiled): one W2_ext matmul per tile =====
            for t in range(NT):
                nc.tensor.matmul(cT_ps[:_gn(t), t, :], _gslice(W2_sb[:], t),
                                 pooled_ext[:], start=True, stop=True)
            compT = sb.tile([128, NT, BL], BF16)
            nc.scalar.copy(compT[:, 0:7, :].rearrange("p t b -> p (t b)"),
                           cT_ps[:, 0:7, :].rearrange("p t b -> p (t b)"))
            nc.vector.tensor_copy(compT[:TAIL, 7, :], cT_ps[:TAIL, 7, :])
            nc.sync.dma_start(out=out_comp_m[:, :, :], in_=compT[:, 0:7, :])
            nc.sync.dma_start(out=out_comp_t[:, :], in_=compT[:TAIL, 7, :])

            # ===== u: comp half =====
            for t in range(NT):
                nc.tensor.matmul(u_ps[:], wc_sb[:_gn(t), t, :], compT[:_gn(t), t, :],
                                 start=False, stop=(t == NT - 1))
            u_sb = sb.tile([H, BL], BF16)
            nc.scalar.activation(u_sb[:], u_ps[:], AF.Copy, scale=inv_sqrt_h)

            # ppi row-sums for the remaining tiles (chunks B / tail)
            for t in range(4, NT):
                prs_tile(t)

            # ===== A/C, pred, LN stats =====
            # predT packs [pred | pred^2] per tile so ONE matmul accumulates
            # both LN statistics (a PSUM bank allows one open group at a time)
            predT = sb.tile([128, NT, 2, BL], F32)

            def ac_tile(t):
                gn = _gn(t)
                so = (t % 2) * 2
                eng = nc.vector if t % 2 == 0 else nc.gpsimd
                ac_ps = cyc([128, 2, BL], name=f"ac{t}")
                nc.tensor.matmul(ac_ps[:gn, 0, :], wgcT[t // 2][:, so, :gn], u_sb[:],
                                 start=True, stop=True)
                nc.tensor.matmul(ac_ps[:gn, 1, :], wgcT[t // 2][:, so + 1, :gn],
                                 u_sb[:], start=True, stop=True)

                bg_t = bgT_bf[:, :, t] if t < NT_MAIN else bgT_tbf[:, :]
                # PSUM-reading ops must stay off gpsimd (HW restriction)
                t1 = work.tile([128, BL], F32, tag="t1")
                nc.vector.tensor_mul(t1[:gn, :], bg_t[:gn], ac_ps[:gn, 0, :])
                t2 = work.tile([128, BL], F32, tag="t2")
                nc.vector.tensor_mul(t2[:gn, :], compT[:gn, t, :], ac_ps[:gn, 1, :])
                nc.vector.tensor_add(t1[:gn, :], t1[:gn, :], t2[:gn, :])
                nc.vector.scalar_tensor_tensor(predT[:gn, t, 0, :], t1[:gn, :],
                                               prs_sb[:gn, t:t + 1], bg_t[:gn],
                                               op0=ALU.add, op1=ALU.mult)
                nc.gpsimd.tensor_mul(predT[:gn, t, 1, :], predT[:gn, t, 0, :],
                                     predT[:gn, t, 0, :])
                nc.tensor.matmul(st_ps[:, :],
                                 ones_col[:gn, :],
                                 predT[:gn, t, :, :].rearrange("p s b -> p (s b)"),
                                 start=(t == 0), stop=(t == NT - 1))

            for t in range(NT):
                ac_tile(t)

            # ===== LayerNorm (rstd via ln+exp: no extra ACT table) =====
            mu = sb.tile([1, BL], F32)
            nc.vector.tensor_scalar_mul(mu[:], st_ps[:, 0:BL], 1.0 / G)
            ex2 = sb.tile([1, BL], F32)
            nc.vector.tensor_scalar_mul(ex2[:], st_ps[:, BL:2 * BL], 1.0 / G)
            mu2 = sb.tile([1, BL], F32)
            nc.vector.tensor_mul(mu2[:], mu[:], mu[:])
            var = sb.tile([1, BL], F32)
            nc.vector.tensor_sub(var[:], ex2[:], mu2[:])
            lv = sb.tile([1, BL], F32)
            nc.scalar.activation(lv[:], var[:], AF.Ln, bias=eps_t[:1, 0:1])
            rstd = sb.tile([1, BL], F32)
            nc.scalar.activation(rstd[:], lv[:], AF.Exp, scale=-0.5)
            mr_ps = cyc([128, 2, BL], name="mr_bc")
            nc.tensor.matmul(mr_ps[:, 0, :], ones_row[:], mu[:], start=True, stop=True)
            nc.tensor.matmul(mr_ps[:, 1, :], ones_row[:], rstd[:], start=True, stop=True)
            rstd_sb = sb.tile([128, BL], F32)
            nc.scalar.copy(rstd_sb[:], mr_ps[:, 1, :])
            # mrs = mu * rstd (broadcast tiles)
            mrs_sb = sb.tile([128, BL], F32)
            nc.vector.tensor_mul(mrs_sb[:], mr_ps[:, 0, :], rstd_sb[:])

            # xn = relu((pred*rstd - mu*rstd)*gam + bet); affine+relu fused
            # into one ACT activation with per-partition scale/bias
            xn = sb.tile([128, NT, BL], BF16)
            for t in range(NT):
                gn = _gn(t)
                eng = nc.gpsimd
                xm = work.tile([128, BL], F32, tag="xm")
                eng.tensor_mul(xm[:gn, :], predT[:gn, t, 0, :], rstd_sb[:gn, :])
                eng.tensor_sub(xm[:gn, :], xm[:gn, :], mrs_sb[:gn, :])
                nc.scalar.activation(xn[:gn, t, :], xm[:gn, :], AF.Relu,
                                     scale=gam_sb[:gn, t:t + 1],
                                     bias=bet_sb[:gn, t:t + 1])

            # ===== FFN (bf16) =====
            NSPLIT = [(0, 512), (512, G - 512)]
            o_ps = [pcyc.tile([BL, n], F32, tag="cyc", name=f"o_ps{i}")
                    for i, (s, n) in enumerate(NSPLIT)]
            for t in range(NT):
                gn = _gn(t)
                for i, (ns, nn) in enumerate(NSPLIT):
                    nc.tensor.matmul(o_ps[i][:], xn[:gn, t, :],
                                     wff_sb[:gn, t, ns:ns + nn],
                                     start=(t == 0), stop=(t == NT - 1))
            pred_out = sb.tile([BL, G], F32)
            nc.scalar.copy(pred_out[:, 0:512], o_ps[0][:])
            nc.vector.tensor_copy(pred_out[:, 512:G], o_ps[1][:])
            nc.sync.dma_start(out=out_pred[:, :], in_=pred_out[:])

    _split_excess_waits(nc)
    return nc


_PER_SAMPLE = ("b_gex", "node_feat", "mask", "adj_matrix", "dist_matrix", "dose", "time")


def kernel(**inputs):
    inputs = {k: np.ascontiguousarray(np.asarray(v, dtype=np.float32))
              for k, v in inputs.items()}
    nc = build_nc()
    in_maps = []
    for c in range(NCORES):
        m = {}
        for k, v in inputs.items():
            if k in _PER_SAMPLE:
                m[k] = np.ascontiguousarray(v[c * BL:(c + 1) * BL])
            else:
                m[k] = v
        in_maps.append(m)
    r = run_bass_kernel_spmd(nc, in_maps, list(range(NCORES)))
    pred = np.concatenate([r.results[c]["out_pred"] for c in range(NCORES)], axis=0)
    comps = []
    for c in range(NCORES):
        cm = np.asarray(r.results[c]["out_comp_m"], np.float32)  # [128, 7, BL]
        ct = np.asarray(r.results[c]["out_comp_t"], np.float32)  # [TAIL, BL]
        full = np.empty((BL, G), np.float32)
        full[:, 0:896] = cm.transpose(2, 0, 1).reshape(BL, 896)
        full[:, 896:G] = ct.T
        comps.append(full)
    comp = np.concatenate(comps, axis=0)
    return pred, comp
